# revision 12
# baseline (speedup 1.0000x reference)
"""Bahdanau-style attention kernel for Trainium2 (8 NeuronCores).

Reference computation (B=32, S=2048, H=1024):
    scores[b,s] = dec[b]@W_dec + enc[b,s]@W_enc + bias      (softmax over s)
    out[b,h]    = sum_s softmax(scores)[b,s] * enc[b,s,h]

Key math fact: softmax over s is shift-invariant, so the per-row constant
dec[b]@W_dec + bias cancels exactly — only enc @ W_enc matters.  Scores are
~N(0, 0.5) for these inputs, so exp() without max-subtraction is safe, and
normalization is deferred to one final scale by 1/sum(exp).

enc is converted to bf16 on the host (tolerance is 2e-2; bf16 end-to-end
error is ~1e-4).  That halves HBM traffic to 16.8 MiB/core (DMA ~47us at
the 358 GB/s per-core roofline), gives the DVE score pass its 2x 16-bit
mode (~38us), and runs the PE weighted-sum matmuls at 1 cycle/row instead
of fp32's 4 (~31us) — so DMA is the single roofline and everything else
hides under it.

Sharding: data-parallel over batch, 4 batches per core; W_enc replicated.

Per-core per-batch dataflow:
  - DMA enc[b] (4 MiB bf16) into SBUF once, as 16 tiles [128s x 1024h]
  - scores: fused multiply+reduce (scalar_tensor_tensor with accum_out) on
    VectorE against a DMA-broadcast bf16 copy of W_enc -> escore [128,16]
    fp32 (tensor_tensor_reduce crashes the device on this runtime; STT is
    the working fused op)
  - exp on ScalarE in groups of 4 columns (fp32 in -> bf16 e out), so PE
    consumption starts mid-batch instead of serializing behind all 16
    scores
  - weighted sum: per tile, 2 PE matmuls (512-col PSUM banks) with the
    bf16 e[:,t] column as stationary, accumulated in PSUM [1,1024] fp32
  - esum = row-sum of e (DVE), total = ones.T @ esum (1-row fp32 matmul),
    out = psum * (1/total) on ScalarE, DMA out
"""

import os
import sys

sys.path.insert(0, "/opt/trn_rl_repo")

import numpy as np
import ml_dtypes

import concourse.bass as bass
import concourse.tile as tile
from concourse import bacc, mybir
from concourse.bass_utils import run_bass_kernel_spmd

B, S, H = 32, 2048, 1024
NCORES = 8
BL = B // NCORES          # 4 batches per core
P = 128                   # SBUF partitions
T = S // P                # 16 s-tiles per batch
F32 = mybir.dt.float32
BF16 = mybir.dt.bfloat16

EXP_G = int(os.environ.get("EXP_G", "4"))   # exp group width (columns)
ENC_BUFS = int(os.environ.get("ENC_BUFS", "48"))
# Score tiles offloaded to GpSimd per batch (DVE takes the rest).  Leave at
# 0: TensorScalarPtr fails the Pool-engine ISA check on core v3, so GpSimd
# cannot run the STT score op at all.
GPN = int(os.environ.get("GPN", "0"))

LAST_RESULTS = None       # test harness introspection


def _build_bass():
    nc = bacc.Bacc("TRN2", target_bir_lowering=False, debug=False)

    enc = nc.dram_tensor("enc", [BL, S, H], BF16, kind="ExternalInput").ap()
    # wenc arrives as a [1, H] bf16 row; broadcast to [P, H] on-chip (a 2 KB
    # DMA + Pool broadcast beats a 256 KB DMA by ~4us of ramp).
    wenc = nc.dram_tensor("wenc", [1, H], BF16, kind="ExternalInput").ap()
    out = nc.dram_tensor("out", [BL, H], F32, kind="ExternalOutput").ap()

    with tile.TileContext(nc) as tc:
        from contextlib import ExitStack

        with ExitStack() as ctx:
            wpool = ctx.enter_context(tc.tile_pool(name="wpool", bufs=1))
            encp = ctx.enter_context(tc.tile_pool(name="encp", bufs=ENC_BUFS))
            scr = ctx.enter_context(tc.tile_pool(name="scr", bufs=4))
            sp = ctx.enter_context(tc.tile_pool(name="sp", bufs=2))
            psp = ctx.enter_context(tc.tile_pool(name="psp", bufs=2, space="PSUM"))

            # wb: tiny row DMA on the scalar HWDGE queue, then Pool broadcast.
            wr = wpool.tile([1, H], BF16, name="wr")
            nc.scalar.dma_start(wr[:], wenc[:])
            wb = wpool.tile([P, H], BF16, name="wb")
            nc.gpsimd.partition_broadcast(wb[:], wr[:])
            ones = wpool.tile([P, 1], F32, name="ones")
            nc.vector.memset(ones[:], 1.0)

            for b in range(BL):
                enc_b = enc[b].rearrange("(t p) h -> t p h", p=P)  # [T,P,H] DRAM view

                tiles = []
                for t in range(T):
                    et = encp.tile([P, H], BF16, name=f"enc_{b}_{t}", tag="enc")
                    if b == 0 and t == 0:
                        # Ramp: a [128,1024] DMA binds its descriptors to one
                        # queue (~4.6us); split tile 0 across 4 queues so the
                        # score pass starts earlier.
                        for i in range(4):
                            nc.sync.dma_start(
                                et[32 * i : 32 * (i + 1), :],
                                enc_b[t][32 * i : 32 * (i + 1), :],
                            )
                    else:
                        nc.sync.dma_start(et[:], enc_b[t])
                    tiles.append(et)

                gp_set = {int((i + 0.5) * T / GPN) for i in range(GPN)}
                escore = sp.tile([P, T], F32, name=f"escore_{b}", tag="escore")
                for t in range(T):
                    eng = nc.gpsimd if t in gp_set else nc.vector
                    stt_out = scr.tile([P, H], BF16, name=f"stt_{b}_{t}", tag="stt")
                    eng.scalar_tensor_tensor(
                        out=stt_out[:],
                        in0=tiles[t][:],
                        scalar=1.0,
                        in1=wb[:],
                        op0=mybir.AluOpType.mult,
                        op1=mybir.AluOpType.mult,
                        accum_out=escore[:, t : t + 1],
                    )

                e = sp.tile([P, T], BF16, name=f"e_{b}", tag="e")
                ps = psp.tile([1, H], F32, name=f"ps_{b}", tag="ps")
                # On the last batch, split the final exp group per-column so
                # the tail after the last score is one matmul pair, not a
                # whole group's worth.
                groups = [(g * EXP_G, (g + 1) * EXP_G) for g in range(T // EXP_G)]
                if b == BL - 1:
                    last = groups.pop()
                    groups += [(t, t + 1) for t in range(last[0], last[1])]
                for lo, hi in groups:
                    nc.scalar.activation(
                        e[:, lo:hi], escore[:, lo:hi],
                        mybir.ActivationFunctionType.Exp,
                    )
                    for t in range(lo, hi):
                        for h0 in (0, 512):
                            nc.tensor.matmul(
                                ps[:, h0 : h0 + 512],
                                lhsT=e[:, t : t + 1],
                                rhs=tiles[t][:, h0 : h0 + 512],
                                start=(t == 0),
                                stop=(t == T - 1),
                            )
                esum = sp.tile([P, 1], F32, name=f"esum_{b}", tag="esum")
                nc.vector.tensor_reduce(
                    esum[:], e[:], axis=mybir.AxisListType.X,
                    op=mybir.AluOpType.add,
                )
                pt = psp.tile([1, 1], F32, name=f"pt_{b}", tag="pt")
                nc.tensor.matmul(pt[:], lhsT=ones[:], rhs=esum[:], start=True, stop=True)
                rtot = sp.tile([1, 1], F32, name=f"rtot_{b}", tag="rtot")
                nc.vector.reciprocal(rtot[:], pt[:])
                ob = sp.tile([1, H], F32, name=f"ob_{b}", tag="ob")
                # Tail: scale + DMA in halves so the first half's output DMA
                # overlaps the second half's scale.
                for h0 in (0, 512):
                    nc.scalar.mul(ob[:, h0 : h0 + 512], ps[:, h0 : h0 + 512], rtot[:])
                    nc.scalar.dma_start(out[b : b + 1, h0 : h0 + 512], ob[:, h0 : h0 + 512])

    nc.compile()
    return nc


_NC_CACHE = None


def kernel(decoder_hidden, encoder_hidden_outputs, W, b):
    global _NC_CACHE, LAST_RESULTS
    enc_full = np.ascontiguousarray(
        np.asarray(encoder_hidden_outputs, dtype=np.float32).astype(ml_dtypes.bfloat16)
    )
    w_enc = np.ascontiguousarray(
        np.asarray(W, dtype=np.float32)[H:, 0].astype(ml_dtypes.bfloat16)[None, :]
    )

    if _NC_CACHE is None:
        _NC_CACHE = _build_bass()
    nc = _NC_CACHE

    in_maps = [
        {"enc": enc_full[i * BL : (i + 1) * BL], "wenc": w_enc}
        for i in range(NCORES)
    ]
    res = run_bass_kernel_spmd(
        nc,
        in_maps,
        core_ids=list(range(NCORES)),
        trace=bool(int(os.environ.get("KERNEL_TRACE", "0"))),
    )
    LAST_RESULTS = res
    out = np.concatenate([res.results[i]["out"] for i in range(NCORES)], axis=0)
    return out.astype(np.float32)


# revision 15
# speedup vs baseline: 1.0414x; 1.0414x over previous
"""Bahdanau-style attention kernel for Trainium2 (8 NeuronCores).

Reference computation (B=32, S=2048, H=1024):
    scores[b,s] = dec[b]@W_dec + enc[b,s]@W_enc + bias      (softmax over s)
    out[b,h]    = sum_s softmax(scores)[b,s] * enc[b,s,h]

Key math fact: softmax over s is shift-invariant, so the per-row constant
dec[b]@W_dec + bias cancels exactly — only enc @ W_enc matters.  Scores are
~N(0, 0.5) for these inputs, so exp() without max-subtraction is safe, and
normalization is deferred to one final scale by 1/sum(exp).

enc is converted to bf16 on the host (tolerance is 2e-2; bf16 end-to-end
error is ~1e-4).  That halves HBM traffic to 16.8 MiB/core (DMA ~47us at
the 358 GB/s per-core roofline), gives the DVE score pass its 2x 16-bit
mode (~38us), and runs the PE weighted-sum matmuls at 1 cycle/row instead
of fp32's 4 (~31us) — so DMA is the single roofline and everything else
hides under it.

Sharding: data-parallel over batch, 4 batches per core; W_enc replicated.

Per-core per-batch dataflow:
  - DMA enc[b] (4 MiB bf16) into SBUF once, as 16 tiles [128s x 1024h]
  - scores: fused multiply+reduce (scalar_tensor_tensor with accum_out) on
    VectorE against a DMA-broadcast bf16 copy of W_enc -> escore [128,16]
    fp32 (tensor_tensor_reduce crashes the device on this runtime; STT is
    the working fused op)
  - exp on ScalarE in groups of 4 columns (fp32 in -> bf16 e out), so PE
    consumption starts mid-batch instead of serializing behind all 16
    scores
  - weighted sum: per tile, 2 PE matmuls (512-col PSUM banks) with the
    bf16 e[:,t] column as stationary, accumulated in PSUM [1,1024] fp32
  - esum = row-sum of e (DVE), total = ones.T @ esum (1-row fp32 matmul),
    out = psum * (1/total) on ScalarE, DMA out
"""

import os
import sys

sys.path.insert(0, "/opt/trn_rl_repo")

import numpy as np
import ml_dtypes

import concourse.bass as bass
import concourse.tile as tile
from concourse import bacc, mybir
from concourse.bass_utils import run_bass_kernel_spmd

B, S, H = 32, 2048, 1024
NCORES = 8
BL = B // NCORES          # 4 batches per core
P = 128                   # SBUF partitions
T = S // P                # 16 s-tiles per batch
F32 = mybir.dt.float32
BF16 = mybir.dt.bfloat16
FP8 = mybir.dt.float8e4

EXP_G = int(os.environ.get("EXP_G", "4"))   # exp group width (columns)
ENC_BUFS = int(os.environ.get("ENC_BUFS", "48"))
# Score tiles offloaded to GpSimd per batch (DVE takes the rest).  Leave at
# 0: TensorScalarPtr fails the Pool-engine ISA check on core v3, so GpSimd
# cannot run the STT score op at all.
GPN = int(os.environ.get("GPN", "0"))

LAST_RESULTS = None       # test harness introspection


def _build_bass():
    nc = bacc.Bacc("TRN2", target_bir_lowering=False, debug=False)

    enc = nc.dram_tensor("enc", [BL, S, H], BF16, kind="ExternalInput").ap()
    # wenc arrives as a [1, H] bf16 row; broadcast to [P, H] on-chip (a 2 KB
    # DMA + Pool broadcast beats a 256 KB DMA by ~4us of ramp).
    wenc = nc.dram_tensor("wenc", [1, H], BF16, kind="ExternalInput").ap()
    out = nc.dram_tensor("out", [BL, H], F32, kind="ExternalOutput").ap()

    with tile.TileContext(nc) as tc:
        from contextlib import ExitStack

        with ExitStack() as ctx:
            wpool = ctx.enter_context(tc.tile_pool(name="wpool", bufs=1))
            encp = ctx.enter_context(tc.tile_pool(name="encp", bufs=ENC_BUFS))
            scr = ctx.enter_context(tc.tile_pool(name="scr", bufs=4))
            sp = ctx.enter_context(tc.tile_pool(name="sp", bufs=2))
            psp = ctx.enter_context(tc.tile_pool(name="psp", bufs=2, space="PSUM"))

            # wb: tiny row DMA first on the sync queue, then Pool broadcast.
            wr = wpool.tile([1, H], BF16, name="wr")
            nc.sync.dma_start(wr[:], wenc[:])
            wb = wpool.tile([P, H], BF16, name="wb")
            nc.gpsimd.partition_broadcast(wb[:], wr[:])
            ones = wpool.tile([P, 1], F32, name="ones")
            nc.vector.memset(ones[:], 1.0)

            for b in range(BL):
                enc_b = enc[b].rearrange("(t p) h -> t p h", p=P)  # [T,P,H] DRAM view

                tiles = []
                for t in range(T):
                    et = encp.tile([P, H], BF16, name=f"enc_{b}_{t}", tag="enc")
                    if b == 0 and t == 0:
                        # Ramp: a [128,1024] DMA binds its descriptors to one
                        # queue (~4.6us); split tile 0 across 4 queues so the
                        # score pass starts earlier.
                        for i in range(4):
                            nc.sync.dma_start(
                                et[32 * i : 32 * (i + 1), :],
                                enc_b[t][32 * i : 32 * (i + 1), :],
                            )
                    else:
                        nc.sync.dma_start(et[:], enc_b[t])
                    tiles.append(et)

                gp_set = {int((i + 0.5) * T / GPN) for i in range(GPN)}
                escore = sp.tile([P, T], F32, name=f"escore_{b}", tag="escore")
                for t in range(T):
                    eng = nc.gpsimd if t in gp_set else nc.vector
                    # stt_out is a discarded side-product (the fp32 accumulator
                    # is the real output); fp8 halves its SBUF write traffic.
                    stt_out = scr.tile([P, H], FP8, name=f"stt_{b}_{t}", tag="stt")
                    eng.scalar_tensor_tensor(
                        out=stt_out[:],
                        in0=tiles[t][:],
                        scalar=1.0,
                        in1=wb[:],
                        op0=mybir.AluOpType.mult,
                        op1=mybir.AluOpType.mult,
                        accum_out=escore[:, t : t + 1],
                    )

                e = sp.tile([P, T], BF16, name=f"e_{b}", tag="e")
                ps = psp.tile([1, H], F32, name=f"ps_{b}", tag="ps")
                # On the last batch, split the final exp group per-column so
                # the tail after the last score is one matmul pair, not a
                # whole group's worth.
                groups = [(g * EXP_G, (g + 1) * EXP_G) for g in range(T // EXP_G)]
                if b == BL - 1:
                    last = groups.pop()
                    groups += [(t, t + 1) for t in range(last[0], last[1])]
                for lo, hi in groups:
                    nc.scalar.activation(
                        e[:, lo:hi], escore[:, lo:hi],
                        mybir.ActivationFunctionType.Exp,
                    )
                    for t in range(lo, hi):
                        for h0 in (0, 512):
                            nc.tensor.matmul(
                                ps[:, h0 : h0 + 512],
                                lhsT=e[:, t : t + 1],
                                rhs=tiles[t][:, h0 : h0 + 512],
                                start=(t == 0),
                                stop=(t == T - 1),
                            )
                esum = sp.tile([P, 1], F32, name=f"esum_{b}", tag="esum")
                nc.vector.tensor_reduce(
                    esum[:], e[:], axis=mybir.AxisListType.X,
                    op=mybir.AluOpType.add,
                )
                pt = psp.tile([1, 1], F32, name=f"pt_{b}", tag="pt")
                nc.tensor.matmul(pt[:], lhsT=ones[:], rhs=esum[:], start=True, stop=True)
                rtot = sp.tile([1, 1], F32, name=f"rtot_{b}", tag="rtot")
                nc.vector.reciprocal(rtot[:], pt[:])
                ob = sp.tile([1, H], F32, name=f"ob_{b}", tag="ob")
                # Tail: scale + DMA in halves so the first half's output DMA
                # overlaps the second half's scale.
                for h0 in (0, 512):
                    nc.scalar.mul(ob[:, h0 : h0 + 512], ps[:, h0 : h0 + 512], rtot[:])
                    nc.scalar.dma_start(out[b : b + 1, h0 : h0 + 512], ob[:, h0 : h0 + 512])

    nc.compile()
    return nc


_NC_CACHE = None


def kernel(decoder_hidden, encoder_hidden_outputs, W, b):
    global _NC_CACHE, LAST_RESULTS
    enc_full = np.ascontiguousarray(
        np.asarray(encoder_hidden_outputs, dtype=np.float32).astype(ml_dtypes.bfloat16)
    )
    w_enc = np.ascontiguousarray(
        np.asarray(W, dtype=np.float32)[H:, 0].astype(ml_dtypes.bfloat16)[None, :]
    )

    if _NC_CACHE is None:
        _NC_CACHE = _build_bass()
    nc = _NC_CACHE

    in_maps = [
        {"enc": enc_full[i * BL : (i + 1) * BL], "wenc": w_enc}
        for i in range(NCORES)
    ]
    res = run_bass_kernel_spmd(
        nc,
        in_maps,
        core_ids=list(range(NCORES)),
        trace=bool(int(os.environ.get("KERNEL_TRACE", "0"))),
    )
    LAST_RESULTS = res
    out = np.concatenate([res.results[i]["out"] for i in range(NCORES)], axis=0)
    return out.astype(np.float32)
